# revision 39
# baseline (speedup 1.0000x reference)
"""Causal multi-head attention on 8 Trainium2 NeuronCores.

Problem: B=2, S=2048, D=1024, H=16, Dh=64 (fp32).
Sharding: core c handles batch b = c//4 and 4 heads [4g, 4g+4), g = c%4
(data parallel over batch x head-group tensor parallel). Each core returns
a partial attention output (its heads' z @ W_O); the host sums the 4 partials
per batch and adds the bias terms.

On-core layout (everything transposed so no on-chip transposes are needed):
  x^T [d, s] comes pre-transposed from the host.
  Q^T, K^T [e, s] per head-pair (head A partitions 0-63, head B 64-127),
  produced by matmul(lhsT=W[d, e2], rhs=x^T[d, s]).
  V [s, e] natural, augmented with a ones column per head so the attention
  V-matmul also produces the softmax denominator.
  scores^T [k, q] = matmul(lhsT=K^T[e,k-tile], rhs=Q^T[e,q-tile]) - the K=64
  contraction auto-packs the two heads of a pair into disjoint PE row groups.
  E = exp(scores^T) with no max subtraction (logits are O(3), exp is safe);
  causal masking multiplies the diagonal blocks by a 0/1 mask slice.
  z^T_aug [65, q] = matmul(lhsT=V_aug[k, 65], rhs=E[k, q]) accumulated over
  k-tiles; row 64 is the softmax denominator. Normalization broadcasts
  recip(denom) across partitions with a K=1 matmul and multiplies on DVE.
  out [s, d] = matmul(lhsT=z^T[e2, s-tile], rhs=W_O[e2, d]) accumulated over
  the two head pairs.

Scale 1/sqrt(Dh) is folded into W_Q/b_Q on the host. b_V's contribution
(sum_h b_V[h] @ W_O[h], constant per row since softmax weights sum to 1)
and b_O are added on the host.
"""

import numpy as np

B, S, D, H, Dh = 2, 2048, 1024, 16, 64
NCORES = 8
CORES_PER_BATCH = 4
HPC = 4          # heads per core (= 2 pairs)
NPAIR = 2
DT_TILES = 8     # 1024 / 128
ST128 = 16       # s tiles of 128
SQ = 512         # q tile width
NQ4 = 4          # q tiles of 512

_BUILT = None


def _build():
    import concourse.bacc as bacc
    import concourse.mybir as mybir
    import concourse.tile as tile

    f32 = mybir.dt.float32
    f32r = mybir.dt.float32r
    EXP = mybir.ActivationFunctionType.Exp

    nc = bacc.Bacc(None)

    xT = nc.dram_tensor("xT", [DT_TILES, 128, S], f32r, kind="ExternalInput")
    wq = nc.dram_tensor("wq", [NPAIR, 128, 1024], f32r, kind="ExternalInput")
    wk = nc.dram_tensor("wk", [NPAIR, 128, 1024], f32r, kind="ExternalInput")
    wv = nc.dram_tensor("wv", [128, 2048], f32r, kind="ExternalInput")
    wo = nc.dram_tensor("wo", [NPAIR, 128, 1024], f32r, kind="ExternalInput")
    bq = nc.dram_tensor("bq", [NPAIR, 128, 1], f32, kind="ExternalInput")
    bk = nc.dram_tensor("bk", [NPAIR, 128, 1], f32, kind="ExternalInput")
    maskd = nc.dram_tensor("maskd", [128, 896], f32r, kind="ExternalInput")
    out = nc.dram_tensor("out", [ST128, 128, 1024], f32, kind="ExternalOutput")

    with tile.TileContext(nc) as tc:
        with (
            tc.tile_pool(name="const", bufs=1) as constp,
            tc.tile_pool(name="qkst", bufs=1) as qkstp,
            tc.tile_pool(name="xchunk", bufs=16) as xp,
            tc.tile_pool(name="work", bufs=3) as workp,
            tc.tile_pool(name="ps", bufs=1, space="PSUM") as ps,
        ):
            # ---- persistent constants / weights ----
            wq_sb = [constp.tile([128, 1024], f32r, tag=f"wq{p}", name=f"wq{p}")
                     for p in range(NPAIR)]
            wk_sb = [constp.tile([128, 1024], f32r, tag=f"wk{p}", name=f"wk{p}")
                     for p in range(NPAIR)]
            wv_sb = constp.tile([128, 2048], f32r, tag="wv", name="wv_sb")
            wo_sb = [constp.tile([128, 1024], f32r, tag=f"wo{p}", name=f"wo{p}")
                     for p in range(NPAIR)]
            bq_sb = [constp.tile([128, 1], f32, tag=f"bq{p}", name=f"bq{p}")
                     for p in range(NPAIR)]
            bk_sb = [constp.tile([128, 1], f32, tag=f"bk{p}", name=f"bk{p}")
                     for p in range(NPAIR)]
            mask_sb = constp.tile([128, 896], f32r, tag="mask", name="mask_sb")
            # all-ones regions of the mask reused as constants:
            # row 0 cols >=384 are 1.0; cols >=511 are 1.0 in every row
            ones_sb = mask_sb[0:1, 384:448]

            for p in range(NPAIR):
                nc.scalar.dma_start(wq_sb[p][:], wq[p])
                nc.scalar.dma_start(wk_sb[p][:], wk[p])
                nc.gpsimd.dma_start(wo_sb[p][:], wo[p])
                nc.gpsimd.dma_start(bq_sb[p][:], bq[p])
                nc.gpsimd.dma_start(bk_sb[p][:], bk[p])
            nc.gpsimd.dma_start(wv_sb[:], wv[:])
            nc.gpsimd.dma_start(mask_sb[:], maskd[:])

            # ---- persistent activations ----
            qt_sb = [qkstp.tile([128, S], f32r, tag=f"qt{p}", name=f"qt{p}")
                     for p in range(NPAIR)]
            kt_sb = [qkstp.tile([128, S], f32r, tag=f"kt{p}", name=f"kt{p}")
                     for p in range(NPAIR)]
            zt_sb = [qkstp.tile([128, S], f32r, tag=f"zt{p}", name=f"zt{p}")
                     for p in range(NPAIR)]
            # V, augmented: head h at cols [65h, 65h+64), ones col at 65h+64
            v_sb = [qkstp.tile([128, 260], f32r, tag=f"v{kt}", name=f"v{kt}")
                    for kt in range(ST128)]
            for kt in range(ST128):
                vo = v_sb[kt].rearrange("p (h c) -> p h c", c=65)
                nc.vector.tensor_copy(vo[:, :, 64], mask_sb[:, 511:515])

            # ---- phase 1 (per 512-wide s block): projections ----
            # returns a list of thunks so the caller can weave projection
            # chunks between attention steps (fills PE bubbles while exp runs)
            def proj_chunks(s4):
                sl = slice(s4 * SQ, (s4 + 1) * SQ)
                xc = []

                def load_x():
                    # spread issue across three DMA-capable queues
                    engs = (nc.sync, nc.sync, nc.sync, nc.scalar, nc.scalar,
                            nc.gpsimd, nc.gpsimd, nc.sync)
                    for t in range(DT_TILES):
                        c = xp.tile([128, SQ], f32r, tag="x", name=f"x{s4}_{t}")
                        engs[t].dma_start(c[:], xT[t][:, sl])
                        xc.append(c)

                def qk_chunk(p):
                    qk_ps = ps.tile([128, 2 * SQ], f32, tag="s", bufs=2,
                                    name=f"qkps{s4}{p}")
                    for t in range(DT_TILES):
                        nc.tensor.matmul(qk_ps[:, 0:SQ], wq_sb[p][:, 128 * t:128 * t + 128], xc[t][:],
                                         start=(t == 0), stop=(t == DT_TILES - 1))
                    for t in range(DT_TILES):
                        nc.tensor.matmul(qk_ps[:, SQ:2 * SQ], wk_sb[p][:, 128 * t:128 * t + 128], xc[t][:],
                                         start=(t == 0), stop=(t == DT_TILES - 1))
                    nc.vector.tensor_scalar_add(qt_sb[p][:, sl], qk_ps[:, 0:SQ],
                                                bq_sb[p][:, 0:1])
                    nc.vector.tensor_scalar_add(kt_sb[p][:, sl], qk_ps[:, SQ:2 * SQ],
                                                bk_sb[p][:, 0:1])

                def v_chunk(j):
                    kt = 4 * s4 + j
                    v_ps = ps.tile([128, 256], f32, tag="v", bufs=2, name=f"vps{kt}")
                    for t in range(DT_TILES):
                        nc.tensor.matmul(v_ps[:],
                                         xc[t][:, j * 128:(j + 1) * 128],
                                         wv_sb[:, 256 * t:256 * t + 256],
                                         start=(t == 0), stop=(t == DT_TILES - 1))
                    for h in range(HPC):
                        nc.vector.tensor_copy(v_sb[kt][:, 65 * h:65 * h + 64],
                                              v_ps[:, 64 * h:64 * h + 64])

                return ([("x", load_x)]
                        + [("qk", lambda p=p: qk_chunk(p)) for p in range(NPAIR)]
                        + [("v", lambda j=j: v_chunk(j)) for j in range(4)])

            # ---- phase 3 (emitted interleaved): output projection ----
            def emit_oproj(q4):
                # the last q-block's projection borrows the idle "v" PSUM tag
                # so it can start while pair 1's z stream still holds "z"
                otag = "v" if q4 == NQ4 - 1 else "z"
                for st in range(4 * q4, 4 * q4 + 4):
                    ssl = slice(st * 128, (st + 1) * 128)
                    for half in range(2):
                        dsl = slice(half * 512, (half + 1) * 512)
                        o_ps = ps.tile([128, 512], f32, tag=otag,
                                       bufs=2, name=f"ops{st}{half}")
                        for p in range(NPAIR):
                            nc.tensor.matmul(o_ps[:], zt_sb[p][:, ssl],
                                             wo_sb[p][:, dsl],
                                             start=(p == 0), stop=(p == NPAIR - 1))
                        o_sb = workp.tile([128, 512], f32, tag="osb",
                                          name=f"osb{st}{half}")
                        nc.vector.tensor_copy(o_sb[:], o_ps[:])
                        nc.sync.dma_start(out[st][:, dsl], o_sb[:])

            # ---- phase 2 (per 512-wide q block): attention ----
            def emit_attn(q4, weave):
                q0 = q4 * SQ
                qsl = slice(q0, q0 + SQ)
                nk = q4 * 4 + 4
                # earlier blocks' projections go at the boundary; the last
                # block's lands in pair 0's normalization window below
                if 0 < q4 < NQ4 - 1:
                    emit_oproj(q4 - 1)
                # x-loads at the head boundary (DMA only); qk chunks at the
                # tail boundary (they share the "s" PSUM tag with the score
                # tiles, so emitting them before the stream would delay it);
                # the v chunks (own tag) weave into the stream as PE filler.
                for t, c in weave:
                    if t == "x":
                        c()
                qks = [c for t, c in weave if t == "qk"]
                weave = [c for t, c in weave if t == "v"]
                steps = NPAIR * nk
                per = max(1, -(-steps // max(1, len(weave))))
                step = 0
                for p in range(NPAIR):
                    za = ps.tile([65, SQ], f32, tag="z", bufs=2, name=f"za{q4}{p}")
                    zb = ps.tile([65, SQ], f32, tag="z", bufs=2, name=f"zb{q4}{p}")
                    for kt in range(nk):
                        ksl = slice(kt * 128, (kt + 1) * 128)
                        s_ps = ps.tile([128, 2 * SQ], f32, tag="s",
                                       bufs=2, name=f"sps{q4}{p}{kt}")
                        nc.tensor.matmul(s_ps[:, 0:SQ],
                                         kt_sb[p][0:64, ksl],
                                         qt_sb[p][0:64, qsl])
                        nc.tensor.matmul(s_ps[:, SQ:2 * SQ],
                                         kt_sb[p][64:128, ksl],
                                         qt_sb[p][64:128, qsl])
                        e = workp.tile([128, 2 * SQ], f32r, tag="e",
                                       bufs=4, name=f"e{q4}{p}{kt}")
                        nc.scalar.activation(e[:], s_ps[:], EXP)
                        d = kt * 128 - q0
                        first = (kt == 0)
                        for sub, zps in ((0, za), (1, zb)):
                            h = 2 * p + sub
                            vap = v_sb[kt][:, 65 * h:65 * h + 65]
                            ebase = sub * SQ
                            if d < 0:  # fully-allowed block
                                nc.tensor.matmul(
                                    zps[:], vap, e[:, ebase:ebase + SQ],
                                    start=first, stop=False,
                                    skip_group_check=True)
                            else:
                                # columns [0, d) fully masked: skip.
                                # columns [d, d+128): mixed - mask-multiply.
                                em = workp.tile([128, 128], f32r, tag="em",
                                                name=f"em{q4}{p}{kt}{sub}")
                                nc.vector.tensor_mul(
                                    em[:], e[:, ebase + d:ebase + d + 128],
                                    mask_sb[:, 384:512])
                                nc.tensor.matmul(
                                    zps[:, d:d + 128], vap, em[:],
                                    start=first, stop=True,
                                    skip_group_check=True)
                                # columns [d+128, 512): fully allowed.
                                if d + 128 < SQ:
                                    nc.tensor.matmul(
                                        zps[:, d + 128:SQ], vap,
                                        e[:, ebase + d + 128:ebase + SQ],
                                        start=first, stop=False,
                                        skip_group_check=True)
                        step += 1
                        if weave and step % per == 0:
                            weave.pop(0)()
                    for sub, zps in ((0, za), (1, zb)):
                        den = workp.tile([1, SQ], f32, tag="den",
                                         name=f"den{q4}{p}{sub}")
                        nc.vector.tensor_copy(den[:], zps[64:65, :])
                        rec = workp.tile([1, SQ], f32, tag="rec",
                                         name=f"rec{q4}{p}{sub}")
                        nc.vector.reciprocal_approx_fast(rec[:], den[:])
                        rb = workp.tile([64, SQ], f32, tag="rb",
                                        name=f"rb{q4}{p}{sub}")
                        nc.gpsimd.partition_broadcast(rb[:], rec[:])
                        nc.vector.tensor_mul(
                            zt_sb[p][64 * sub:64 * sub + 64, qsl],
                            zps[0:64, :], rb[:])
                    # fill the last block's pair-0 normalization window with
                    # the previous block's output projection
                    if p == 0 and q4 == NQ4 - 1 and q4 > 0:
                        emit_oproj(q4 - 1)
                while weave:
                    weave.pop(0)()
                for c in qks:
                    c()

            # fully interleave: projections for x-block s4+1 are woven into
            # the attention stream of q-block s4 (whose deps only reach s4),
            # so the PSUM tag rotation flows without a phase barrier and
            # matmul-dense projection chunks fill exp-bound PE bubbles.
            for _, c in proj_chunks(0):
                c()
            for q4 in range(NQ4):
                nxt = proj_chunks(q4 + 1) if q4 + 1 < NQ4 else []
                emit_attn(q4, nxt)
            emit_oproj(NQ4 - 1)

    nc.compile()
    return nc


def _get_built():
    global _BUILT
    if _BUILT is None:
        _BUILT = _build()
    return _BUILT


def _host_prep(x, W_Q, W_K, W_V, W_O, b_Q, b_K):
    """Build the 8 per-core input maps."""
    scale = np.float32(1.0 / np.sqrt(Dh))
    mask = (np.arange(896)[None, :] >= (np.arange(128)[:, None] + 384)
            ).astype(np.float32)
    in_maps = []
    for c in range(NCORES):
        b = c // CORES_PER_BATCH
        g = c % CORES_PER_BATCH
        hs = slice(HPC * g, HPC * g + HPC)
        xT_b = np.ascontiguousarray(x[b].T).reshape(DT_TILES, 128, S)
        def pack_de(w):
            # [4 heads, D, Dh] -> pair-stacked [2, D, 128] -> [2, 128, 8*128]
            a = w.reshape(NPAIR, 2, D, Dh).transpose(0, 2, 1, 3).reshape(
                NPAIR, DT_TILES, 128, 128)
            return np.ascontiguousarray(a.transpose(0, 2, 1, 3)).reshape(
                NPAIR, 128, 1024)

        wq_c = pack_de(W_Q[hs] * scale)
        wk_c = pack_de(W_K[hs])
        wv_c = np.ascontiguousarray(
            W_V[hs].transpose(1, 0, 2).reshape(DT_TILES, 128, HPC * Dh)
            .transpose(1, 0, 2)).reshape(128, 2048)
        wo_c = np.ascontiguousarray(W_O[hs]).reshape(NPAIR, 128, 1024)
        bq_c = np.ascontiguousarray(b_Q[hs] * scale).reshape(NPAIR, 128, 1)
        bk_c = np.ascontiguousarray(b_K[hs]).reshape(NPAIR, 128, 1)
        in_maps.append({
            "xT": xT_b, "wq": wq_c, "wk": wk_c, "wv": wv_c, "wo": wo_c,
            "bq": bq_c, "bk": bk_c, "maskd": mask,
        })
    return in_maps


def kernel(normalized_resid_pre, W_Q, W_K, W_V, W_O, b_Q, b_K, b_V, b_O,
           _want_profile=False):
    from concourse.bass_utils import run_bass_kernel_spmd

    x = np.asarray(normalized_resid_pre, np.float32)
    W_Q = np.asarray(W_Q, np.float32)
    W_K = np.asarray(W_K, np.float32)
    W_V = np.asarray(W_V, np.float32)
    W_O = np.asarray(W_O, np.float32)
    b_Q = np.asarray(b_Q, np.float32)
    b_K = np.asarray(b_K, np.float32)
    b_V = np.asarray(b_V, np.float32)
    b_O = np.asarray(b_O, np.float32)

    in_maps = _host_prep(x, W_Q, W_K, W_V, W_O, b_Q, b_K)
    nc = _get_built()
    kw = {}
    if _want_profile:
        kw = dict(trace=True)
    res = run_bass_kernel_spmd(nc, in_maps, list(range(NCORES)), **kw)

    # host-side unshard: sum the head-group partials per batch + bias terms
    b_eff = b_O + np.einsum("he,hed->d", b_V, W_O).astype(np.float32)
    attn_out = np.zeros((B, S, D), np.float32)
    for c in range(NCORES):
        b = c // CORES_PER_BATCH
        attn_out[b] += res.results[c]["out"].reshape(S, D)
    attn_out += b_eff[None, None, :]
    if _want_profile:
        return attn_out, res
    return attn_out


# revision 41
# speedup vs baseline: 1.1092x; 1.1092x over previous
"""Causal multi-head attention on 8 Trainium2 NeuronCores.

Problem: B=2, S=2048, D=1024, H=16, Dh=64 (fp32).
Sharding: core c handles batch b = c//4 and 4 heads [4g, 4g+4), g = c%4
(data parallel over batch x head-group tensor parallel). Each core returns
a partial attention output (its heads' z @ W_O); the host sums the 4 partials
per batch and adds the bias terms.

On-core layout (everything transposed so no on-chip transposes are needed):
  x^T [d, s] comes pre-transposed from the host.
  Q^T, K^T [e, s] per head-pair (head A partitions 0-63, head B 64-127),
  produced by matmul(lhsT=W[d, e2], rhs=x^T[d, s]).
  V [s, e] natural, augmented with a ones column per head so the attention
  V-matmul also produces the softmax denominator.
  scores^T [k, q] = matmul(lhsT=K^T[e,k-tile], rhs=Q^T[e,q-tile]) - the K=64
  contraction auto-packs the two heads of a pair into disjoint PE row groups.
  E = exp(scores^T) with no max subtraction (logits are O(3), exp is safe);
  causal masking multiplies the diagonal blocks by a 0/1 mask slice.
  z^T_aug [65, q] = matmul(lhsT=V_aug[k, 65], rhs=E[k, q]) accumulated over
  k-tiles; row 64 is the softmax denominator. Normalization broadcasts
  recip(denom) across partitions with a K=1 matmul and multiplies on DVE.
  out [s, d] = matmul(lhsT=z^T[e2, s-tile], rhs=W_O[e2, d]) accumulated over
  the two head pairs.

Scale 1/sqrt(Dh) is folded into W_Q/b_Q on the host. b_V's contribution
(sum_h b_V[h] @ W_O[h], constant per row since softmax weights sum to 1)
and b_O are added on the host.
"""

import numpy as np

B, S, D, H, Dh = 2, 2048, 1024, 16, 64
NCORES = 8
CORES_PER_BATCH = 4
HPC = 4          # heads per core (= 2 pairs)
NPAIR = 2
DT_TILES = 8     # 1024 / 128
ST128 = 16       # s tiles of 128
SQ = 512         # q tile width
NQ4 = 4          # q tiles of 512

_BUILT = None


def _build():
    import concourse.bacc as bacc
    import concourse.mybir as mybir
    import concourse.tile as tile

    f32 = mybir.dt.float32
    f32r = mybir.dt.float32r
    EXP = mybir.ActivationFunctionType.Exp

    nc = bacc.Bacc(None)

    xT = nc.dram_tensor("xT", [DT_TILES, 128, S], f32r, kind="ExternalInput")
    wq = nc.dram_tensor("wq", [NPAIR, 128, 1024], f32r, kind="ExternalInput")
    wk = nc.dram_tensor("wk", [NPAIR, 128, 1024], f32r, kind="ExternalInput")
    wv = nc.dram_tensor("wv", [128, 2048], f32r, kind="ExternalInput")
    wo = nc.dram_tensor("wo", [NPAIR, 128, 1024], f32r, kind="ExternalInput")
    bq = nc.dram_tensor("bq", [NPAIR, 128, 1], f32, kind="ExternalInput")
    bk = nc.dram_tensor("bk", [NPAIR, 128, 1], f32, kind="ExternalInput")
    maskd = nc.dram_tensor("maskd", [128, 896], f32r, kind="ExternalInput")
    out = nc.dram_tensor("out", [ST128, 128, 1024], f32, kind="ExternalOutput")

    with tile.TileContext(nc) as tc:
        with (
            tc.tile_pool(name="const", bufs=1) as constp,
            tc.tile_pool(name="qkst", bufs=1) as qkstp,
            tc.tile_pool(name="xchunk", bufs=16) as xp,
            tc.tile_pool(name="work", bufs=3) as workp,
            tc.tile_pool(name="ps", bufs=1, space="PSUM") as ps,
        ):
            # ---- persistent constants / weights ----
            wq_sb = [constp.tile([128, 1024], f32r, tag=f"wq{p}", name=f"wq{p}")
                     for p in range(NPAIR)]
            wk_sb = [constp.tile([128, 1024], f32r, tag=f"wk{p}", name=f"wk{p}")
                     for p in range(NPAIR)]
            wv_sb = constp.tile([128, 2048], f32r, tag="wv", name="wv_sb")
            wo_sb = [constp.tile([128, 1024], f32r, tag=f"wo{p}", name=f"wo{p}")
                     for p in range(NPAIR)]
            bq_sb = [constp.tile([128, 1], f32, tag=f"bq{p}", name=f"bq{p}")
                     for p in range(NPAIR)]
            bk_sb = [constp.tile([128, 1], f32, tag=f"bk{p}", name=f"bk{p}")
                     for p in range(NPAIR)]
            mask_sb = constp.tile([128, 896], f32r, tag="mask", name="mask_sb")
            # all-ones regions of the mask reused as constants:
            # row 0 cols >=384 are 1.0; cols >=511 are 1.0 in every row
            ones_sb = mask_sb[0:1, 384:448]

            for p in range(NPAIR):
                nc.scalar.dma_start(wq_sb[p][:], wq[p])
                nc.scalar.dma_start(wk_sb[p][:], wk[p])
                nc.gpsimd.dma_start(wo_sb[p][:], wo[p])
                nc.gpsimd.dma_start(bq_sb[p][:], bq[p])
                nc.gpsimd.dma_start(bk_sb[p][:], bk[p])
            nc.gpsimd.dma_start(wv_sb[:], wv[:])
            nc.gpsimd.dma_start(mask_sb[:], maskd[:])

            # ---- persistent activations ----
            qt_sb = [qkstp.tile([128, S], f32r, tag=f"qt{p}", name=f"qt{p}")
                     for p in range(NPAIR)]
            kt_sb = [qkstp.tile([128, S], f32r, tag=f"kt{p}", name=f"kt{p}")
                     for p in range(NPAIR)]
            zt_sb = [qkstp.tile([128, S], f32r, tag=f"zt{p}", name=f"zt{p}")
                     for p in range(NPAIR)]
            # V, augmented: head h at cols [65h, 65h+64), ones col at 65h+64
            v_sb = [qkstp.tile([128, 260], f32r, tag=f"v{kt}", name=f"v{kt}")
                    for kt in range(ST128)]
            for kt in range(ST128):
                vo = v_sb[kt].rearrange("p (h c) -> p h c", c=65)
                nc.vector.tensor_copy(vo[:, :, 64], mask_sb[:, 511:515])

            # ---- phase 1 (per 512-wide s block): projections ----
            # returns a list of thunks so the caller can weave projection
            # chunks between attention steps (fills PE bubbles while exp runs)
            def proj_chunks(s4):
                sl = slice(s4 * SQ, (s4 + 1) * SQ)
                xc = []

                def load_x():
                    # spread issue across three DMA-capable queues
                    engs = (nc.sync, nc.sync, nc.sync, nc.scalar, nc.scalar,
                            nc.gpsimd, nc.gpsimd, nc.sync)
                    for t in range(DT_TILES):
                        c = xp.tile([128, SQ], f32r, tag="x", name=f"x{s4}_{t}")
                        engs[t].dma_start(c[:], xT[t][:, sl])
                        xc.append(c)

                def qk_chunk(p):
                    qk_ps = ps.tile([128, 2 * SQ], f32, tag="s", bufs=2,
                                    name=f"qkps{s4}{p}")
                    for t in range(DT_TILES):
                        nc.tensor.matmul(qk_ps[:, 0:SQ], wq_sb[p][:, 128 * t:128 * t + 128], xc[t][:],
                                         start=(t == 0), stop=(t == DT_TILES - 1))
                    for t in range(DT_TILES):
                        nc.tensor.matmul(qk_ps[:, SQ:2 * SQ], wk_sb[p][:, 128 * t:128 * t + 128], xc[t][:],
                                         start=(t == 0), stop=(t == DT_TILES - 1))
                    nc.vector.tensor_scalar_add(qt_sb[p][:, sl], qk_ps[:, 0:SQ],
                                                bq_sb[p][:, 0:1])
                    nc.vector.tensor_scalar_add(kt_sb[p][:, sl], qk_ps[:, SQ:2 * SQ],
                                                bk_sb[p][:, 0:1])

                def v_chunk(j):
                    kt = 4 * s4 + j
                    v_ps = ps.tile([128, 256], f32, tag="v", bufs=2, name=f"vps{kt}")
                    for t in range(DT_TILES):
                        nc.tensor.matmul(v_ps[:],
                                         xc[t][:, j * 128:(j + 1) * 128],
                                         wv_sb[:, 256 * t:256 * t + 256],
                                         start=(t == 0), stop=(t == DT_TILES - 1))
                    for h in range(HPC):
                        nc.vector.tensor_copy(v_sb[kt][:, 65 * h:65 * h + 64],
                                              v_ps[:, 64 * h:64 * h + 64])

                return ([("x", load_x)]
                        + [("qk", lambda p=p: qk_chunk(p)) for p in range(NPAIR)]
                        + [("v", lambda j=j: v_chunk(j)) for j in range(4)])

            # ---- phase 3 (emitted interleaved): output projection ----
            def emit_oproj(q4):
                # the last q-block's projection borrows the idle "v" PSUM tag
                # so it can start while pair 1's z stream still holds "z"
                otag = "v" if q4 == NQ4 - 1 else "z"
                for st in range(4 * q4, 4 * q4 + 4):
                    ssl = slice(st * 128, (st + 1) * 128)
                    for half in range(2):
                        dsl = slice(half * 512, (half + 1) * 512)
                        o_ps = ps.tile([128, 512], f32, tag=otag,
                                       bufs=2, name=f"ops{st}{half}")
                        for p in range(NPAIR):
                            nc.tensor.matmul(o_ps[:], zt_sb[p][:, ssl],
                                             wo_sb[p][:, dsl],
                                             start=(p == 0), stop=(p == NPAIR - 1))
                        o_sb = workp.tile([128, 512], f32, tag="osb",
                                          name=f"osb{st}{half}")
                        nc.vector.tensor_copy(o_sb[:], o_ps[:])
                        nc.sync.dma_start(out[st][:, dsl], o_sb[:])

            # ---- phase 2 (per 512-wide q block): attention ----
            def emit_attn(q4, weave):
                q0 = q4 * SQ
                qsl = slice(q0, q0 + SQ)
                nk = q4 * 4 + 4
                # earlier blocks' projections go at the boundary; the last
                # block's lands in pair 0's normalization window below
                if 0 < q4 < NQ4 - 1:
                    emit_oproj(q4 - 1)
                # x-load and qk chunks go at the head boundary (emitting the
                # next block's qk before this stream keeps the s-tag slot
                # rotation seamless across the stream-to-stream transition);
                # the v chunks (own tag) weave into the stream as PE filler.
                while weave and weave[0][0] != "v":
                    weave.pop(0)[1]()
                weave = [c for t, c in weave]
                steps = NPAIR * nk
                per = max(1, -(-steps // max(1, len(weave))))
                step = 0
                for p in range(NPAIR):
                    za = ps.tile([65, SQ], f32, tag="z", bufs=2, name=f"za{q4}{p}")
                    zb = ps.tile([65, SQ], f32, tag="z", bufs=2, name=f"zb{q4}{p}")
                    for kt in range(nk):
                        ksl = slice(kt * 128, (kt + 1) * 128)
                        s_ps = ps.tile([128, 2 * SQ], f32, tag="s",
                                       bufs=2, name=f"sps{q4}{p}{kt}")
                        nc.tensor.matmul(s_ps[:, 0:SQ],
                                         kt_sb[p][0:64, ksl],
                                         qt_sb[p][0:64, qsl])
                        nc.tensor.matmul(s_ps[:, SQ:2 * SQ],
                                         kt_sb[p][64:128, ksl],
                                         qt_sb[p][64:128, qsl])
                        e = workp.tile([128, 2 * SQ], f32r, tag="e",
                                       bufs=4, name=f"e{q4}{p}{kt}")
                        nc.scalar.activation(e[:], s_ps[:], EXP)
                        d = kt * 128 - q0
                        first = (kt == 0)
                        for sub, zps in ((0, za), (1, zb)):
                            h = 2 * p + sub
                            vap = v_sb[kt][:, 65 * h:65 * h + 65]
                            ebase = sub * SQ
                            if d < 0:  # fully-allowed block
                                nc.tensor.matmul(
                                    zps[:], vap, e[:, ebase:ebase + SQ],
                                    start=first, stop=False,
                                    skip_group_check=True)
                            else:
                                # columns [0, d) fully masked: skip.
                                # columns [d, d+128): mixed - mask-multiply.
                                em = workp.tile([128, 128], f32r, tag="em",
                                                name=f"em{q4}{p}{kt}{sub}")
                                nc.vector.tensor_mul(
                                    em[:], e[:, ebase + d:ebase + d + 128],
                                    mask_sb[:, 384:512])
                                nc.tensor.matmul(
                                    zps[:, d:d + 128], vap, em[:],
                                    start=first, stop=True,
                                    skip_group_check=True)
                                # columns [d+128, 512): fully allowed.
                                if d + 128 < SQ:
                                    nc.tensor.matmul(
                                        zps[:, d + 128:SQ], vap,
                                        e[:, ebase + d + 128:ebase + SQ],
                                        start=first, stop=False,
                                        skip_group_check=True)
                        step += 1
                        if weave and step % per == 0:
                            weave.pop(0)()
                    for sub, zps in ((0, za), (1, zb)):
                        den = workp.tile([1, SQ], f32, tag="den",
                                         name=f"den{q4}{p}{sub}")
                        nc.vector.tensor_copy(den[:], zps[64:65, :])
                        rec = workp.tile([1, SQ], f32, tag="rec",
                                         name=f"rec{q4}{p}{sub}")
                        nc.vector.reciprocal_approx_fast(rec[:], den[:])
                        rb = workp.tile([64, SQ], f32, tag="rb",
                                        name=f"rb{q4}{p}{sub}")
                        nc.gpsimd.partition_broadcast(rb[:], rec[:])
                        nc.vector.tensor_mul(
                            zt_sb[p][64 * sub:64 * sub + 64, qsl],
                            zps[0:64, :], rb[:])
                    # fill the last block's pair-0 normalization window with
                    # the previous block's output projection
                    if p == 0 and q4 == NQ4 - 1 and q4 > 0:
                        emit_oproj(q4 - 1)
                while weave:
                    weave.pop(0)()

            # fully interleave: projections for x-block s4+1 are woven into
            # the attention stream of q-block s4 (whose deps only reach s4),
            # so the PSUM tag rotation flows without a phase barrier and
            # matmul-dense projection chunks fill exp-bound PE bubbles.
            for _, c in proj_chunks(0):
                c()
            for q4 in range(NQ4):
                nxt = proj_chunks(q4 + 1) if q4 + 1 < NQ4 else []
                emit_attn(q4, nxt)
            emit_oproj(NQ4 - 1)

    nc.compile()
    return nc


def _get_built():
    global _BUILT
    if _BUILT is None:
        _BUILT = _build()
    return _BUILT


def _host_prep(x, W_Q, W_K, W_V, W_O, b_Q, b_K):
    """Build the 8 per-core input maps."""
    scale = np.float32(1.0 / np.sqrt(Dh))
    mask = (np.arange(896)[None, :] >= (np.arange(128)[:, None] + 384)
            ).astype(np.float32)
    in_maps = []
    for c in range(NCORES):
        b = c // CORES_PER_BATCH
        g = c % CORES_PER_BATCH
        hs = slice(HPC * g, HPC * g + HPC)
        xT_b = np.ascontiguousarray(x[b].T).reshape(DT_TILES, 128, S)
        def pack_de(w):
            # [4 heads, D, Dh] -> pair-stacked [2, D, 128] -> [2, 128, 8*128]
            a = w.reshape(NPAIR, 2, D, Dh).transpose(0, 2, 1, 3).reshape(
                NPAIR, DT_TILES, 128, 128)
            return np.ascontiguousarray(a.transpose(0, 2, 1, 3)).reshape(
                NPAIR, 128, 1024)

        wq_c = pack_de(W_Q[hs] * scale)
        wk_c = pack_de(W_K[hs])
        wv_c = np.ascontiguousarray(
            W_V[hs].transpose(1, 0, 2).reshape(DT_TILES, 128, HPC * Dh)
            .transpose(1, 0, 2)).reshape(128, 2048)
        wo_c = np.ascontiguousarray(W_O[hs]).reshape(NPAIR, 128, 1024)
        bq_c = np.ascontiguousarray(b_Q[hs] * scale).reshape(NPAIR, 128, 1)
        bk_c = np.ascontiguousarray(b_K[hs]).reshape(NPAIR, 128, 1)
        in_maps.append({
            "xT": xT_b, "wq": wq_c, "wk": wk_c, "wv": wv_c, "wo": wo_c,
            "bq": bq_c, "bk": bk_c, "maskd": mask,
        })
    return in_maps


def kernel(normalized_resid_pre, W_Q, W_K, W_V, W_O, b_Q, b_K, b_V, b_O,
           _want_profile=False):
    from concourse.bass_utils import run_bass_kernel_spmd

    x = np.asarray(normalized_resid_pre, np.float32)
    W_Q = np.asarray(W_Q, np.float32)
    W_K = np.asarray(W_K, np.float32)
    W_V = np.asarray(W_V, np.float32)
    W_O = np.asarray(W_O, np.float32)
    b_Q = np.asarray(b_Q, np.float32)
    b_K = np.asarray(b_K, np.float32)
    b_V = np.asarray(b_V, np.float32)
    b_O = np.asarray(b_O, np.float32)

    in_maps = _host_prep(x, W_Q, W_K, W_V, W_O, b_Q, b_K)
    nc = _get_built()
    kw = {}
    if _want_profile:
        kw = dict(trace=True)
    res = run_bass_kernel_spmd(nc, in_maps, list(range(NCORES)), **kw)

    # host-side unshard: sum the head-group partials per batch + bias terms
    b_eff = b_O + np.einsum("he,hed->d", b_V, W_O).astype(np.float32)
    attn_out = np.zeros((B, S, D), np.float32)
    for c in range(NCORES):
        b = c // CORES_PER_BATCH
        attn_out[b] += res.results[c]["out"].reshape(S, D)
    attn_out += b_eff[None, None, :]
    if _want_profile:
        return attn_out, res
    return attn_out


# revision 42
# speedup vs baseline: 1.1144x; 1.0046x over previous
"""Causal multi-head attention on 8 Trainium2 NeuronCores.

Problem: B=2, S=2048, D=1024, H=16, Dh=64 (fp32).
Sharding: core c handles batch b = c//4 and 4 heads [4g, 4g+4), g = c%4
(data parallel over batch x head-group tensor parallel). Each core returns
a partial attention output (its heads' z @ W_O); the host sums the 4 partials
per batch and adds the bias terms.

On-core layout (everything transposed so no on-chip transposes are needed):
  x^T [d, s] comes pre-transposed from the host.
  Q^T, K^T [e, s] per head-pair (head A partitions 0-63, head B 64-127),
  produced by matmul(lhsT=W[d, e2], rhs=x^T[d, s]).
  V [s, e] natural, augmented with a ones column per head so the attention
  V-matmul also produces the softmax denominator.
  scores^T [k, q] = matmul(lhsT=K^T[e,k-tile], rhs=Q^T[e,q-tile]) - the K=64
  contraction auto-packs the two heads of a pair into disjoint PE row groups.
  E = exp(scores^T) with no max subtraction (logits are O(3), exp is safe);
  causal masking multiplies the diagonal blocks by a 0/1 mask slice.
  z^T_aug [65, q] = matmul(lhsT=V_aug[k, 65], rhs=E[k, q]) accumulated over
  k-tiles; row 64 is the softmax denominator. Normalization broadcasts
  recip(denom) across partitions with a K=1 matmul and multiplies on DVE.
  out [s, d] = matmul(lhsT=z^T[e2, s-tile], rhs=W_O[e2, d]) accumulated over
  the two head pairs.

Scale 1/sqrt(Dh) is folded into W_Q/b_Q on the host. b_V's contribution
(sum_h b_V[h] @ W_O[h], constant per row since softmax weights sum to 1)
and b_O are added on the host.
"""

import numpy as np

B, S, D, H, Dh = 2, 2048, 1024, 16, 64
NCORES = 8
CORES_PER_BATCH = 4
HPC = 4          # heads per core (= 2 pairs)
NPAIR = 2
DT_TILES = 8     # 1024 / 128
ST128 = 16       # s tiles of 128
SQ = 512         # q tile width
NQ4 = 4          # q tiles of 512

_BUILT = None


def _build():
    import concourse.bacc as bacc
    import concourse.mybir as mybir
    import concourse.tile as tile

    f32 = mybir.dt.float32
    f32r = mybir.dt.float32r
    EXP = mybir.ActivationFunctionType.Exp

    nc = bacc.Bacc(None)

    xT = nc.dram_tensor("xT", [DT_TILES, 128, S], f32r, kind="ExternalInput")
    wq = nc.dram_tensor("wq", [NPAIR, 128, 1024], f32r, kind="ExternalInput")
    wk = nc.dram_tensor("wk", [NPAIR, 128, 1024], f32r, kind="ExternalInput")
    wv = nc.dram_tensor("wv", [128, 2048], f32r, kind="ExternalInput")
    wo = nc.dram_tensor("wo", [NPAIR, 128, 1024], f32r, kind="ExternalInput")
    bq = nc.dram_tensor("bq", [NPAIR, 128, 1], f32, kind="ExternalInput")
    bk = nc.dram_tensor("bk", [NPAIR, 128, 1], f32, kind="ExternalInput")
    maskd = nc.dram_tensor("maskd", [128, 896], f32r, kind="ExternalInput")
    out = nc.dram_tensor("out", [ST128, 128, 1024], f32, kind="ExternalOutput")

    with tile.TileContext(nc) as tc:
        with (
            tc.tile_pool(name="const", bufs=1) as constp,
            tc.tile_pool(name="qkst", bufs=1) as qkstp,
            tc.tile_pool(name="xchunk", bufs=16) as xp,
            tc.tile_pool(name="work", bufs=3) as workp,
            tc.tile_pool(name="ps", bufs=1, space="PSUM") as ps,
        ):
            # ---- persistent constants / weights ----
            wq_sb = [constp.tile([128, 1024], f32r, tag=f"wq{p}", name=f"wq{p}")
                     for p in range(NPAIR)]
            wk_sb = [constp.tile([128, 1024], f32r, tag=f"wk{p}", name=f"wk{p}")
                     for p in range(NPAIR)]
            wv_sb = constp.tile([128, 2048], f32r, tag="wv", name="wv_sb")
            wo_sb = [constp.tile([128, 1024], f32r, tag=f"wo{p}", name=f"wo{p}")
                     for p in range(NPAIR)]
            bq_sb = [constp.tile([128, 1], f32, tag=f"bq{p}", name=f"bq{p}")
                     for p in range(NPAIR)]
            bk_sb = [constp.tile([128, 1], f32, tag=f"bk{p}", name=f"bk{p}")
                     for p in range(NPAIR)]
            mask_sb = constp.tile([128, 896], f32r, tag="mask", name="mask_sb")
            # all-ones regions of the mask reused as constants:
            # row 0 cols >=384 are 1.0; cols >=511 are 1.0 in every row
            ones_sb = mask_sb[0:1, 384:448]

            for p in range(NPAIR):
                nc.scalar.dma_start(wq_sb[p][:], wq[p])
                nc.scalar.dma_start(wk_sb[p][:], wk[p])
                nc.gpsimd.dma_start(bq_sb[p][:], bq[p])
                nc.gpsimd.dma_start(bk_sb[p][:], bk[p])
            nc.gpsimd.dma_start(wv_sb[:], wv[:])
            nc.gpsimd.dma_start(mask_sb[:], maskd[:])

            # ---- persistent activations ----
            qt_sb = [qkstp.tile([128, S], f32r, tag=f"qt{p}", name=f"qt{p}")
                     for p in range(NPAIR)]
            kt_sb = [qkstp.tile([128, S], f32r, tag=f"kt{p}", name=f"kt{p}")
                     for p in range(NPAIR)]
            zt_sb = [qkstp.tile([128, S], f32r, tag=f"zt{p}", name=f"zt{p}")
                     for p in range(NPAIR)]
            # V, augmented: head h at cols [65h, 65h+64), ones col at 65h+64
            v_sb = [qkstp.tile([128, 260], f32r, tag=f"v{kt}", name=f"v{kt}")
                    for kt in range(ST128)]
            for kt in range(ST128):
                vo = v_sb[kt].rearrange("p (h c) -> p h c", c=65)
                nc.vector.tensor_copy(vo[:, :, 64], mask_sb[:, 511:515])

            # ---- phase 1 (per 512-wide s block): projections ----
            # returns a list of thunks so the caller can weave projection
            # chunks between attention steps (fills PE bubbles while exp runs)
            def proj_chunks(s4):
                sl = slice(s4 * SQ, (s4 + 1) * SQ)
                xc = []

                def load_x():
                    for t in range(DT_TILES):
                        c = xp.tile([128, SQ], f32r, tag="x", name=f"x{s4}_{t}")
                        nc.sync.dma_start(c[:], xT[t][:, sl])
                        xc.append(c)

                def qk_chunk(p):
                    qk_ps = ps.tile([128, 2 * SQ], f32, tag="s", bufs=2,
                                    name=f"qkps{s4}{p}")
                    for t in range(DT_TILES):
                        nc.tensor.matmul(qk_ps[:, 0:SQ], wq_sb[p][:, 128 * t:128 * t + 128], xc[t][:],
                                         start=(t == 0), stop=(t == DT_TILES - 1))
                    for t in range(DT_TILES):
                        nc.tensor.matmul(qk_ps[:, SQ:2 * SQ], wk_sb[p][:, 128 * t:128 * t + 128], xc[t][:],
                                         start=(t == 0), stop=(t == DT_TILES - 1))
                    nc.vector.tensor_scalar_add(qt_sb[p][:, sl], qk_ps[:, 0:SQ],
                                                bq_sb[p][:, 0:1])
                    nc.vector.tensor_scalar_add(kt_sb[p][:, sl], qk_ps[:, SQ:2 * SQ],
                                                bk_sb[p][:, 0:1])

                def v_chunk(j):
                    kt = 4 * s4 + j
                    v_ps = ps.tile([128, 256], f32, tag="v", bufs=2, name=f"vps{kt}")
                    for t in range(DT_TILES):
                        nc.tensor.matmul(v_ps[:],
                                         xc[t][:, j * 128:(j + 1) * 128],
                                         wv_sb[:, 256 * t:256 * t + 256],
                                         start=(t == 0), stop=(t == DT_TILES - 1))
                    for h in range(HPC):
                        nc.vector.tensor_copy(v_sb[kt][:, 65 * h:65 * h + 64],
                                              v_ps[:, 64 * h:64 * h + 64])

                return ([("x", load_x)]
                        + [("qk", lambda p=p: qk_chunk(p)) for p in range(NPAIR)]
                        + [("v", lambda j=j: v_chunk(j)) for j in range(4)])

            # ---- phase 3 (emitted interleaved): output projection ----
            def emit_oproj(q4):
                # the last q-block's projection borrows the idle "v" PSUM tag
                # so it can start while pair 1's z stream still holds "z"
                otag = "v" if q4 == NQ4 - 1 else "z"
                for st in range(4 * q4, 4 * q4 + 4):
                    ssl = slice(st * 128, (st + 1) * 128)
                    for half in range(2):
                        dsl = slice(half * 512, (half + 1) * 512)
                        o_ps = ps.tile([128, 512], f32, tag=otag,
                                       bufs=2, name=f"ops{st}{half}")
                        for p in range(NPAIR):
                            nc.tensor.matmul(o_ps[:], zt_sb[p][:, ssl],
                                             wo_sb[p][:, dsl],
                                             start=(p == 0), stop=(p == NPAIR - 1))
                        o_sb = workp.tile([128, 512], f32, tag="osb",
                                          name=f"osb{st}{half}")
                        nc.vector.tensor_copy(o_sb[:], o_ps[:])
                        nc.sync.dma_start(out[st][:, dsl], o_sb[:])

            # ---- phase 2 (per 512-wide q block): attention ----
            def emit_attn(q4, weave):
                q0 = q4 * SQ
                qsl = slice(q0, q0 + SQ)
                nk = q4 * 4 + 4
                # earlier blocks' projections go at the boundary; the last
                # block's lands in pair 0's normalization window below
                if 0 < q4 < NQ4 - 1:
                    emit_oproj(q4 - 1)
                # x-load and qk chunks go at the head boundary (emitting the
                # next block's qk before this stream keeps the s-tag slot
                # rotation seamless across the stream-to-stream transition);
                # the v chunks (own tag) weave into the stream as PE filler.
                while weave and weave[0][0] != "v":
                    weave.pop(0)[1]()
                weave = [c for t, c in weave]
                steps = NPAIR * nk
                per = max(1, -(-steps // max(1, len(weave))))
                step = 0
                for p in range(NPAIR):
                    za = ps.tile([65, SQ], f32, tag="z", bufs=2, name=f"za{q4}{p}")
                    zb = ps.tile([65, SQ], f32, tag="z", bufs=2, name=f"zb{q4}{p}")
                    for kt in range(nk):
                        ksl = slice(kt * 128, (kt + 1) * 128)
                        s_ps = ps.tile([128, 2 * SQ], f32, tag="s",
                                       bufs=2, name=f"sps{q4}{p}{kt}")
                        nc.tensor.matmul(s_ps[:, 0:SQ],
                                         kt_sb[p][0:64, ksl],
                                         qt_sb[p][0:64, qsl])
                        nc.tensor.matmul(s_ps[:, SQ:2 * SQ],
                                         kt_sb[p][64:128, ksl],
                                         qt_sb[p][64:128, qsl])
                        e = workp.tile([128, 2 * SQ], f32r, tag="e",
                                       bufs=4, name=f"e{q4}{p}{kt}")
                        nc.scalar.activation(e[:], s_ps[:], EXP)
                        d = kt * 128 - q0
                        first = (kt == 0)
                        for sub, zps in ((0, za), (1, zb)):
                            h = 2 * p + sub
                            vap = v_sb[kt][:, 65 * h:65 * h + 65]
                            ebase = sub * SQ
                            if d < 0:  # fully-allowed block
                                nc.tensor.matmul(
                                    zps[:], vap, e[:, ebase:ebase + SQ],
                                    start=first, stop=False,
                                    skip_group_check=True)
                            else:
                                # columns [0, d) fully masked: skip.
                                # columns [d, d+128): mixed - mask-multiply.
                                em = workp.tile([128, 128], f32r, tag="em",
                                                name=f"em{q4}{p}{kt}{sub}")
                                nc.vector.tensor_mul(
                                    em[:], e[:, ebase + d:ebase + d + 128],
                                    mask_sb[:, 384:512])
                                nc.tensor.matmul(
                                    zps[:, d:d + 128], vap, em[:],
                                    start=first, stop=True,
                                    skip_group_check=True)
                                # columns [d+128, 512): fully allowed.
                                if d + 128 < SQ:
                                    nc.tensor.matmul(
                                        zps[:, d + 128:SQ], vap,
                                        e[:, ebase + d + 128:ebase + SQ],
                                        start=first, stop=False,
                                        skip_group_check=True)
                        step += 1
                        if weave and step % per == 0:
                            weave.pop(0)()
                    for sub, zps in ((0, za), (1, zb)):
                        den = workp.tile([1, SQ], f32, tag="den",
                                         name=f"den{q4}{p}{sub}")
                        nc.vector.tensor_copy(den[:], zps[64:65, :])
                        rec = workp.tile([1, SQ], f32, tag="rec",
                                         name=f"rec{q4}{p}{sub}")
                        nc.vector.reciprocal_approx_fast(rec[:], den[:])
                        rb = workp.tile([64, SQ], f32, tag="rb",
                                        name=f"rb{q4}{p}{sub}")
                        nc.gpsimd.partition_broadcast(rb[:], rec[:])
                        nc.vector.tensor_mul(
                            zt_sb[p][64 * sub:64 * sub + 64, qsl],
                            zps[0:64, :], rb[:])
                    # fill the last block's pair-0 normalization window with
                    # the previous block's output projection
                    if p == 0 and q4 == NQ4 - 1 and q4 > 0:
                        emit_oproj(q4 - 1)
                while weave:
                    weave.pop(0)()

            # fully interleave: projections for x-block s4+1 are woven into
            # the attention stream of q-block s4 (whose deps only reach s4),
            # so the PSUM tag rotation flows without a phase barrier and
            # matmul-dense projection chunks fill exp-bound PE bubbles.
            for _, c in proj_chunks(0):
                c()
            for p in range(NPAIR):  # W_O not needed until the first o-proj
                nc.gpsimd.dma_start(wo_sb[p][:], wo[p])
            for q4 in range(NQ4):
                nxt = proj_chunks(q4 + 1) if q4 + 1 < NQ4 else []
                emit_attn(q4, nxt)
            emit_oproj(NQ4 - 1)

    nc.compile()
    return nc


def _get_built():
    global _BUILT
    if _BUILT is None:
        _BUILT = _build()
    return _BUILT


def _host_prep(x, W_Q, W_K, W_V, W_O, b_Q, b_K):
    """Build the 8 per-core input maps."""
    scale = np.float32(1.0 / np.sqrt(Dh))
    mask = (np.arange(896)[None, :] >= (np.arange(128)[:, None] + 384)
            ).astype(np.float32)
    in_maps = []
    for c in range(NCORES):
        b = c // CORES_PER_BATCH
        g = c % CORES_PER_BATCH
        hs = slice(HPC * g, HPC * g + HPC)
        xT_b = np.ascontiguousarray(x[b].T).reshape(DT_TILES, 128, S)
        def pack_de(w):
            # [4 heads, D, Dh] -> pair-stacked [2, D, 128] -> [2, 128, 8*128]
            a = w.reshape(NPAIR, 2, D, Dh).transpose(0, 2, 1, 3).reshape(
                NPAIR, DT_TILES, 128, 128)
            return np.ascontiguousarray(a.transpose(0, 2, 1, 3)).reshape(
                NPAIR, 128, 1024)

        wq_c = pack_de(W_Q[hs] * scale)
        wk_c = pack_de(W_K[hs])
        wv_c = np.ascontiguousarray(
            W_V[hs].transpose(1, 0, 2).reshape(DT_TILES, 128, HPC * Dh)
            .transpose(1, 0, 2)).reshape(128, 2048)
        wo_c = np.ascontiguousarray(W_O[hs]).reshape(NPAIR, 128, 1024)
        bq_c = np.ascontiguousarray(b_Q[hs] * scale).reshape(NPAIR, 128, 1)
        bk_c = np.ascontiguousarray(b_K[hs]).reshape(NPAIR, 128, 1)
        in_maps.append({
            "xT": xT_b, "wq": wq_c, "wk": wk_c, "wv": wv_c, "wo": wo_c,
            "bq": bq_c, "bk": bk_c, "maskd": mask,
        })
    return in_maps


def kernel(normalized_resid_pre, W_Q, W_K, W_V, W_O, b_Q, b_K, b_V, b_O,
           _want_profile=False):
    from concourse.bass_utils import run_bass_kernel_spmd

    x = np.asarray(normalized_resid_pre, np.float32)
    W_Q = np.asarray(W_Q, np.float32)
    W_K = np.asarray(W_K, np.float32)
    W_V = np.asarray(W_V, np.float32)
    W_O = np.asarray(W_O, np.float32)
    b_Q = np.asarray(b_Q, np.float32)
    b_K = np.asarray(b_K, np.float32)
    b_V = np.asarray(b_V, np.float32)
    b_O = np.asarray(b_O, np.float32)

    in_maps = _host_prep(x, W_Q, W_K, W_V, W_O, b_Q, b_K)
    nc = _get_built()
    kw = {}
    if _want_profile:
        kw = dict(trace=True)
    res = run_bass_kernel_spmd(nc, in_maps, list(range(NCORES)), **kw)

    # host-side unshard: sum the head-group partials per batch + bias terms
    b_eff = b_O + np.einsum("he,hed->d", b_V, W_O).astype(np.float32)
    attn_out = np.zeros((B, S, D), np.float32)
    for c in range(NCORES):
        b = c // CORES_PER_BATCH
        attn_out[b] += res.results[c]["out"].reshape(S, D)
    attn_out += b_eff[None, None, :]
    if _want_profile:
        return attn_out, res
    return attn_out
